# revision 11
# baseline (speedup 1.0000x reference)
"""Expert-parallel MoE FFN (ChronosMOEFeedForward) for 8 Trainium2 cores.

Strategy (sharding_hint: expert-parallel):
  - Router (softmax + top-2 over E=16 experts) computed on host in fp32 —
    top-k decisions must match the fp32 reference's ordering, and the router
    GEMM is ~0.1% of total FLOPs.
  - The 16 experts are sharded 2-per-core across 8 cores. Tokens routed to
    each expert are gathered on host (the "all-to-all dispatch"), padded to a
    fixed capacity C, and shipped transposed as [H, C] so the device GEMM
    chain needs no on-device transposes.
  - Per core the device computes, per expert e:
        gT = Wg[e].T @ XeT           [I, C]   (bf16 inputs, fp32 PSUM accum)
        uT = Wu[e].T @ XeT           [I, C]
        aT = silu(gT) * uT * w_bcast [I, C]   (combine weight broadcast on-chip)
        yT = Wd[e].T @ aT            [H, C]   (tokens stay on the free dim)
  - Host scatters each expert's y rows back to the owning tokens ("combine").
    A token's two expert contributions land in two disjoint slot arrays
    (top-1 slot, top-2 slot), so the combine is collision-free fancy
    indexing plus one add — no np.add.at.

Each PSUM group is computed with its k-tiles contiguous (all 16/8
accumulating matmuls back-to-back into one bank) — consecutive matmuls that
alternate PSUM banks measure ~40ns/MM slower on HW than k-contiguous
streams. The 8-deep PSUM pool overlaps each group's eviction with the next
group's matmuls, and the wg/wu halves share a 2-slot pool so the next
expert's weights stream in while the current expert computes. y returns as
bf16 to halve the output DMA.

The dense reference formulation computes all 16 experts for every token;
routed top-2 computes only 2 — an 8x FLOP reduction, plus bf16 matmuls with
fp32 PSUM accumulation.
"""

import numpy as np
import ml_dtypes

import concourse.mybir as mybir
import concourse.tile as tile
from concourse import bacc
from concourse.bass_utils import run_bass_kernel_spmd

# Problem shapes (hardcoded per contract).
H = 2048        # hidden size
I = 1024        # moe intermediate size
E = 16          # num experts
TOPK = 2
B, S = 4, 1024
T = B * S       # 4096 tokens
N_CORES = 8
EPC = E // N_CORES  # experts per core = 2
# Per-slot token capacities: each core gets one "heavy" expert (slot 0) and
# one "light" expert (slot 1). The host assigns the 8 heaviest-loaded experts
# to slot 0. Mean load is 512 +- ~22, so the light half of the experts almost
# always fits in 512; the few tokens that overflow fall back to exact numpy.
CAPS = (640, 512)
C = CAPS[0]      # max capacity (DRAM params are padded to this)
NCHS = (320, 256)  # free-dim chunk per slot for the g/u matmuls (2 chunks)

BF16 = ml_dtypes.bfloat16

KT_H = H // 128  # 16 k-tiles over H
MT_I = I // 128  # 8 m-tiles over I
KT_I = I // 128  # 8 k-tiles over I

# Split-precision fp8 phase A: host ships wg/wu and x as fp8e4 high+low
# pairs; the device computes the three significant half-products
# (wh*xh + wl*xh + wh*xl) with DoubleRow matmuls (2 fp8 MACs/cell/cycle),
# cutting phase-A PE cycles ~15% vs bf16 while keeping ~bf16 accuracy
# (the dropped wl*xl term is ~0.4% of each product). Phase B stays bf16.
FP8A = True
F8 = ml_dtypes.float8_e4m3  # IEEE e4m3 (bias 7, +-240) = TRN FP8_EXP4
WSCALE = 64.0  # weights ~0.02*N(0,1) sit in e4m3 subnormals unless scaled

_CACHE = {}


def _build_nc(caps=CAPS, loop_r=None, internal=False):
    """Build the per-core Bass module (SPMD: all cores run this program).

    caps: per-slot token capacities. kernel() derives them from the actual
    routing (phase A streams tokens on the matmul free dim, so capacity is
    not 128-quantized there) and caches one compiled module per caps value.
    loop_r/internal are for the timing harness only: Internal DRAM I/O (no
    host transfers) with the body repeated loop_r times on-device.
    """
    import contextlib

    nc = bacc.Bacc(None, target_bir_lowering=False)
    f32 = mybir.dt.float32
    bf16 = mybir.dt.bfloat16
    f8 = mybir.dt.float8e4

    def par(name, shape, dt):
        if internal:
            return nc.dram_tensor(name, shape, dt)
        return nc.declare_dram_parameter(name, shape, dt, isOutput=False)

    if FP8A:
        ghl = par("ghl", [EPC, 2 * H, I], f8)   # (wh_t, wl_t) k-tile pairs
        gh = par("gh", [EPC, H, I], f8)
        uhl = par("uhl", [EPC, 2 * H, I], f8)
        uh = par("uh", [EPC, H, I], f8)
        xhh = par("xhh", [EPC, 2 * H, C], f8)   # xh k-tiles duplicated
        xl = par("xl", [EPC, H, C], f8)
        ins = (ghl, gh, uhl, uh, xhh, xl)
    else:
        xg = par("xg", [EPC, H, C], bf16)
        gww = par("gww", [EPC, H, I], bf16)
        uww = par("uww", [EPC, H, I], bf16)
        ins = (xg, gww, uww)
    wdp = par("wdp", [EPC, I, H], bf16)
    wtv = par("wtv", [EPC, C], f32)
    if internal:
        y = nc.dram_tensor("y", [EPC, H, C], bf16)
        done = nc.declare_dram_parameter("done", [1, 1], f32, isOutput=True)
    else:
        y = nc.declare_dram_parameter("y", [EPC, H, C], bf16, isOutput=True)

    with tile.TileContext(nc) as tc:
        with (
            tc.tile_pool(name="wpool", bufs=2) as wpool,   # wg/wu halves share slots
            tc.tile_pool(name="xpool", bufs=1) as xpool,
            tc.tile_pool(name="wdpool", bufs=1) as wdpool,
            tc.tile_pool(name="apool", bufs=1) as apool,
            tc.tile_pool(name="small", bufs=2) as small,
            tc.tile_pool(name="yp", bufs=4) as yp,
            tc.tile_pool(name="ps", bufs=8, space="PSUM") as ps,
        ):
            const = small.tile([1, 128], f32, tag="ones")
            nc.any.memset(const[:], 1.0)

            loop_cm = (
                tc.For_i(0, loop_r, 1) if loop_r else contextlib.nullcontext()
            )
            with loop_cm:
                if FP8A:
                    _emit_body_fp8(nc, tc, caps, ins, wdp, wtv, y, const,
                                   wpool, xpool, wdpool, apool, small, yp, ps)
                else:
                    _emit_body(nc, tc, caps, ins[0], ins[1], ins[2], wdp, wtv,
                               y, const, wpool, xpool, wdpool, apool, small,
                               yp, ps)

            if internal:
                dn = small.tile([1, 1], f32, tag="done")
                nc.any.memset(dn[:], 1.0)
                nc.sync.dma_start(out=done[:], in_=dn[:])

    nc.compile()
    return nc


def _emit_body_fp8(nc, tc, caps, ins, wdp, wtv, y, const,
                   wpool, xpool, wdpool, apool, small, yp, ps):
    """Phase A with split-precision fp8 DoubleRow; phase B bf16 (as before).

    Per (matrix, m-tile, chunk) the PSUM group accumulates 24 DoubleRow MMs:
    16 over (wh,wl)-pair k-tiles vs duplicated xh, then 8 over wh-pair
    k-tiles vs xl pairs — computing (wh+wl)*xh + wh*xl ~ w*x. Weights are
    pre-scaled by WSCALE on host (e4m3 range); sigmoid descales via the
    activation's scale input and the combine weights absorb 1/WSCALE^2.
    """
    f32 = mybir.dt.float32
    bf16 = mybir.dt.bfloat16
    f8 = mybir.dt.float8e4
    DR = mybir.MatmulPerfMode.DoubleRow
    ghl, gh, uhl, uh, xhh, xl = ins
    for e in range(EPC):
        Ce = caps[e]
        NCH = Ce if Ce <= 512 else (Ce + 1) // 2
        wt_sb = small.tile([1, C], f32, tag="wt")
        nc.sync.dma_start(out=wt_sb[:, :Ce], in_=wtv[e][None, :Ce])
        # g-side streams + x streams, in first-use order
        ghl_sb = wpool.tile([128, 2 * KT_H, I], f8, tag="whl")
        xhh_sb = xpool.tile([128, 2 * KT_H, C], f8, tag="xhh")
        for ko in range(2 * KT_H):
            nc.sync.dma_start(
                out=ghl_sb[:, ko, :], in_=ghl[e, ko * 128 : (ko + 1) * 128, :]
            )
            nc.sync.dma_start(
                out=xhh_sb[:, ko, :Ce],
                in_=xhh[e, ko * 128 : (ko + 1) * 128, :Ce],
            )
        gh_sb = wpool.tile([128, KT_H, I], f8, tag="wh8")
        xl_sb = xpool.tile([128, KT_H, C], f8, tag="xl8")
        for ko in range(KT_H):
            nc.sync.dma_start(
                out=gh_sb[:, ko, :], in_=gh[e, ko * 128 : (ko + 1) * 128, :]
            )
            nc.sync.dma_start(
                out=xl_sb[:, ko, :Ce], in_=xl[e, ko * 128 : (ko + 1) * 128, :Ce]
            )
        uhl_sb = wpool.tile([128, 2 * KT_H, I], f8, tag="whl")
        for ko in range(2 * KT_H):
            nc.sync.dma_start(
                out=uhl_sb[:, ko, :], in_=uhl[e, ko * 128 : (ko + 1) * 128, :]
            )
        uh_sb = wpool.tile([128, KT_H, I], f8, tag="wh8")
        for ko in range(KT_H):
            nc.sync.dma_start(
                out=uh_sb[:, ko, :], in_=uh[e, ko * 128 : (ko + 1) * 128, :]
            )
        wd_sb = wdpool.tile([128, KT_I, H], bf16, tag="wd")
        for ko in range(KT_I):
            nc.sync.dma_start(
                out=wd_sb[:, ko, :], in_=wdp[e, ko * 128 : (ko + 1) * 128, :]
            )

        sg_sb = apool.tile([128, MT_I, C], bf16, tag="sg")
        a_sb = apool.tile([128, MT_I, C], bf16, tag="a")
        wbc_sb = small.tile([128, C], bf16, tag="wbc")

        for c0 in range(0, Ce, NCH):
            w = min(NCH, Ce - c0)
            pw = ps.tile([128, 512], f32, tag="ps")
            nc.tensor.matmul(
                pw[:, :w], lhsT=const[:], rhs=wt_sb[:, c0 : c0 + w],
                start=True, stop=True,
            )
            nc.vector.tensor_copy(wbc_sb[:, c0 : c0 + w], pw[:, :w])

        # ---- phase A (fp8 DoubleRow, k-contiguous per psum group)
        for mat in range(2):
            whl_sb = ghl_sb if mat == 0 else uhl_sb
            wh8_sb = gh_sb if mat == 0 else uh_sb
            for m in range(MT_I):
                for c0 in range(0, Ce, NCH):
                    w = min(NCH, Ce - c0)
                    pt_t = ps.tile([128, 512], f32, tag="ps", name=f"ps_{m}_{c0}")
                    for t in range(KT_H):
                        nc.tensor.matmul(
                            pt_t[:, :w],
                            lhsT=whl_sb[:, 2 * t : 2 * t + 2, m * 128 : (m + 1) * 128],
                            rhs=xhh_sb[:, 2 * t : 2 * t + 2, c0 : c0 + w],
                            start=(t == 0),
                            stop=False,
                            perf_mode=DR,
                        )
                    for t in range(KT_H // 2):
                        nc.tensor.matmul(
                            pt_t[:, :w],
                            lhsT=wh8_sb[:, 2 * t : 2 * t + 2, m * 128 : (m + 1) * 128],
                            rhs=xl_sb[:, 2 * t : 2 * t + 2, c0 : c0 + w],
                            start=False,
                            stop=(t == KT_H // 2 - 1),
                            perf_mode=DR,
                        )
                    pt = pt_t[:, :w]
                    if mat == 0:
                        sig = small.tile([128, NCH], bf16, tag="sig")
                        nc.scalar.activation(
                            sig[:, :w], pt, mybir.ActivationFunctionType.Sigmoid,
                            scale=1.0 / WSCALE,
                        )
                        nc.vector.tensor_mul(sg_sb[:, m, c0 : c0 + w], sig[:, :w], pt)
                    else:
                        tmp = small.tile([128, NCH], bf16, tag="tmp")
                        nc.vector.tensor_mul(tmp[:, :w], sg_sb[:, m, c0 : c0 + w], pt)
                        nc.vector.tensor_mul(
                            a_sb[:, m, c0 : c0 + w], tmp[:, :w], wbc_sb[:, c0 : c0 + w]
                        )

        # ---- phase B (bf16, unchanged)
        bchunks = [(0, Ce)] if Ce <= 512 else [(0, NCH), (NCH, Ce - NCH)]
        for m in range(H // 128):
            yt = yp.tile([128, C], bf16, tag="ysb", name=f"yt_{m % 4}")
            for c0, w in bchunks:
                bpt = ps.tile([128, 512], f32, tag="ps", name=f"bps_{m % 4}_{c0}")
                for k in range(KT_I):
                    nc.tensor.matmul(
                        bpt[:, :w],
                        lhsT=wd_sb[:, k, m * 128 : (m + 1) * 128],
                        rhs=a_sb[:, k, c0 : c0 + w],
                        start=(k == 0),
                        stop=(k == KT_I - 1),
                    )
                nc.vector.tensor_copy(yt[:, c0 : c0 + w], bpt[:, :w])
            nc.sync.dma_start(
                out=y[e, m * 128 : (m + 1) * 128, :Ce], in_=yt[:, :Ce]
            )


def _emit_body(nc, tc, caps, xg, gww, uww, wdp, wtv, y, const,
               wpool, xpool, wdpool, apool, small, yp, ps):
    f32 = mybir.dt.float32
    bf16 = mybir.dt.bfloat16
    if True:
        if True:
            for e in range(EPC):
                Ce = caps[e]
                # one free-dim chunk if it fits a PSUM bank, else an even split
                NCH = Ce if Ce <= 512 else (Ce + 1) // 2
                # DMA issue order = need order: combine weights (tiny, feeds
                # the first PE instr group placed later), wg+xg k-tiles
                # interleaved (phase A ramp), then wu, then wd (phase B).
                wt_sb = small.tile([1, C], f32, tag="wt")
                nc.sync.dma_start(out=wt_sb[:, :Ce], in_=wtv[e][None, :Ce])
                gw_sb = wpool.tile([128, KT_H, I], bf16, tag="guw")
                xg_sb = xpool.tile([128, KT_H, C], bf16, tag="xg")
                for ko in range(KT_H):
                    nc.sync.dma_start(
                        out=gw_sb[:, ko, :], in_=gww[e, ko * 128 : (ko + 1) * 128, :]
                    )
                    nc.sync.dma_start(
                        out=xg_sb[:, ko, :Ce],
                        in_=xg[e, ko * 128 : (ko + 1) * 128, :Ce],
                    )
                uw_sb = wpool.tile([128, KT_H, I], bf16, tag="guw")
                for ko in range(KT_H):
                    nc.sync.dma_start(
                        out=uw_sb[:, ko, :], in_=uww[e, ko * 128 : (ko + 1) * 128, :]
                    )
                wd_sb = wdpool.tile([128, KT_I, H], bf16, tag="wd")
                for ko in range(KT_I):
                    nc.sync.dma_start(
                        out=wd_sb[:, ko, :], in_=wdp[e, ko * 128 : (ko + 1) * 128, :]
                    )

                sg_sb = apool.tile([128, MT_I, C], bf16, tag="sg")
                a_sb = apool.tile([128, MT_I, C], bf16, tag="a")
                wbc_sb = small.tile([128, C], bf16, tag="wbc")

                # broadcast combine weights across partitions via outer
                # product ones[128] x wt[C] -> wbc[128, C]; runs inside the
                # initial DMA ramp (wt is the first DMA issued) before any
                # g-group claims a psum bank
                for c0 in range(0, Ce, NCH):
                    w = min(NCH, Ce - c0)
                    pw = ps.tile([128, 512], f32, tag="ps")
                    nc.tensor.matmul(
                        pw[:, :w],
                        lhsT=const[:],
                        rhs=wt_sb[:, c0 : c0 + w],
                        start=True,
                        stop=True,
                    )
                    nc.vector.tensor_copy(wbc_sb[:, c0 : c0 + w], pw[:, :w])

                # ---- phase A: gT/uT, one k-contiguous psum group at a time.
                # Consecutive MMs that alternate psum banks pay ~40ns each on
                # HW (measured); k-contiguous streams run at ~ideal, and the
                # 8-deep psum pool still overlaps each group's eviction with
                # the next group's MMs.
                for mat in range(2):  # 0: g (silu), 1: u (mul + weight)
                    w_sb = gw_sb if mat == 0 else uw_sb
                    for m in range(MT_I):
                        for c0 in range(0, Ce, NCH):
                            w = min(NCH, Ce - c0)
                            pt_t = ps.tile([128, 512], f32, tag="ps", name=f"ps_{m}_{c0}")
                            for k in range(KT_H):
                                nc.tensor.matmul(
                                    pt_t[:, :w],
                                    lhsT=w_sb[:, k, m * 128 : (m + 1) * 128],
                                    rhs=xg_sb[:, k, c0 : c0 + w],
                                    start=(k == 0),
                                    stop=(k == KT_H - 1),
                                )
                            pt = pt_t[:, :w]
                            if mat == 0:
                                # silu(g) = g * sigmoid(g)
                                sig = small.tile([128, NCH], bf16, tag="sig")
                                nc.scalar.activation(
                                    sig[:, :w], pt, mybir.ActivationFunctionType.Sigmoid
                                )
                                nc.vector.tensor_mul(
                                    sg_sb[:, m, c0 : c0 + w], sig[:, :w], pt
                                )
                            else:
                                tmp = small.tile([128, NCH], bf16, tag="tmp")
                                nc.vector.tensor_mul(
                                    tmp[:, :w], sg_sb[:, m, c0 : c0 + w], pt
                                )
                                nc.vector.tensor_mul(
                                    a_sb[:, m, c0 : c0 + w],
                                    tmp[:, :w],
                                    wbc_sb[:, c0 : c0 + w],
                                )

                # ---- phase B: yT = Wd.T @ a   [H, Ce] — tokens stay on the
                # free dim, so only the exact Ce columns are streamed (no
                # 128-row quantization like the y = a.T @ Wd layout)
                bchunks = (
                    [(0, Ce)] if Ce <= 512 else [(0, NCH), (NCH, Ce - NCH)]
                )
                for m in range(H // 128):  # m-tiles over H, k-contiguous
                    yt = yp.tile([128, C], bf16, tag="ysb", name=f"yt_{m % 4}")
                    for c0, w in bchunks:
                        bpt = ps.tile(
                            [128, 512], f32, tag="ps", name=f"bps_{m % 4}_{c0}"
                        )
                        for k in range(KT_I):
                            nc.tensor.matmul(
                                bpt[:, :w],
                                lhsT=wd_sb[:, k, m * 128 : (m + 1) * 128],
                                rhs=a_sb[:, k, c0 : c0 + w],
                                start=(k == 0),
                                stop=(k == KT_I - 1),
                            )
                        nc.vector.tensor_copy(yt[:, c0 : c0 + w], bpt[:, :w])
                    nc.sync.dma_start(
                        out=y[e, m * 128 : (m + 1) * 128, :Ce],
                        in_=yt[:, :Ce],
                    )


def _route(xf, gate_w):
    """Top-2 routing, mirroring the fp32 reference semantics exactly."""
    logits = xf @ gate_w.T.astype(np.float32)          # [T, E]
    logits -= logits.max(axis=-1, keepdims=True)
    scores = np.exp(logits)
    scores /= scores.sum(axis=-1, keepdims=True)
    i1 = scores.argmax(axis=-1)
    s1 = scores[np.arange(T), i1]
    masked = scores.copy()
    masked[np.arange(T), i1] = -np.inf
    i2 = masked.argmax(axis=-1)
    s2 = scores[np.arange(T), i2]
    denom = s1 + s2 + 1e-20
    return i1, s1 / denom, i2, s2 / denom


def _expert_np(xrows, wts, wg_e, wu_e, wd_e):
    """Exact fp32 fallback for capacity-overflow tokens (rare)."""
    g = xrows @ wg_e
    u = xrows @ wu_e
    a = (g / (1.0 + np.exp(-g))) * u * wts[:, None]
    return a @ wd_e


def _pack(xf, gate_w, wg, wu, wd):
    """Route + gather + pack per-core device inputs.

    Experts are assigned to (core, slot) by load: the 8 heaviest go to the
    C=640 slot 0, the 8 lightest to the C=512 slot 1. The assignment is pure
    host-side data placement — the SPMD program is identical on every core.
    """
    i1, w1, i2, w2 = _route(xf, gate_w)
    per_e = []
    for e in range(E):
        l1 = np.nonzero(i1 == e)[0]
        l2 = np.nonzero(i2 == e)[0]
        toks = np.concatenate([l1, l2])
        wts = np.concatenate([w1[l1], w2[l2]])
        ranks = np.concatenate(
            [np.zeros(len(l1), np.int8), np.ones(len(l2), np.int8)]
        )
        per_e.append((toks, ranks, wts))
    loads = [len(pe[0]) for pe in per_e]
    order = np.argsort([-n for n in loads], kind="stable")
    # exact capacities from this routing: slot 0 covers the heaviest expert
    # (up to the 640 DRAM padding), slot 1 stays at <=512 so its phase-B tile
    # count stays at 4; the rare overflow tokens go to the exact numpy path
    caps = (
        min(loads[order[0]], CAPS[0]),
        min(max(loads[order[N_CORES]], 128), CAPS[1]),
    )

    in_maps = []
    tok_lists = []
    for c in range(N_CORES):
        wtc = np.zeros((EPC, C), np.float32)
        core_toks = []
        experts = [int(order[c]), int(order[2 * N_CORES - 1 - c])]
        if FP8A:
            ghl = np.zeros((EPC, 2 * H, I), F8)
            ghp = np.zeros((EPC, H, I), F8)
            uhl = np.zeros((EPC, 2 * H, I), F8)
            uhp = np.zeros((EPC, H, I), F8)
            xhh = np.zeros((EPC, 2 * H, C), F8)
            xlp = np.zeros((EPC, H, C), F8)
        else:
            xgc = np.zeros((EPC, H, C), BF16)
        for j in range(EPC):
            e = experts[j]
            toks, ranks, wts = per_e[e]
            n_dev = min(len(toks), caps[j])
            if FP8A:
                xe = np.ascontiguousarray(xf[toks[:n_dev]].T)      # [H, n]
                xh = xe.astype(F8)
                xr = (xe - xh.astype(np.float32)).astype(F8)
                xhh[j, :, :n_dev] = _ilv(xh, xh)
                xlp[j, :, :n_dev] = xr
                for dsthl, dsth, wmat in ((ghl, ghp, wg), (uhl, uhp, wu)):
                    ws = wmat[e] * WSCALE                          # [H, I]
                    wh = ws.astype(F8)
                    wl = (ws - wh.astype(np.float32)).astype(F8)
                    dsthl[j] = _ilv(wh, wl)
                    dsth[j] = wh
                wtc[j, :n_dev] = wts[:n_dev] / (WSCALE * WSCALE)
            else:
                xgc[j, :, :n_dev] = xf[toks[:n_dev]].T.astype(BF16)
                wtc[j, :n_dev] = wts[:n_dev]
            core_toks.append((toks, ranks, wts, n_dev, e))
        tok_lists.append(core_toks)
        im = {"wdp": wd[experts].astype(BF16), "wtv": wtc}
        if FP8A:
            im.update(ghl=ghl, gh=ghp, uhl=uhl, uh=uhp, xhh=xhh, xl=xlp)
        else:
            im.update(
                xg=xgc,
                gww=wg[experts].astype(BF16),
                uww=wu[experts].astype(BF16),
            )
        in_maps.append(im)
    return in_maps, tok_lists, caps


def _ilv(a, b):
    """Interleave the 128-row k-tiles of a and b: [H', X] x2 -> [2H', X]."""
    kt = a.shape[0] // 128
    st = np.stack([a.reshape(kt, 128, -1), b.reshape(kt, 128, -1)], axis=1)
    return st.reshape(2 * a.shape[0], a.shape[1])


def kernel(x, gate_w, wg, wu, wd):
    # inputs may arrive as jax arrays (immutable) — normalize to numpy first
    x = np.asarray(x)
    gate_w = np.asarray(gate_w, dtype=np.float32)
    in_dtype = x.dtype
    xf = np.ascontiguousarray(x.reshape(T, H), dtype=np.float32)
    wg = np.asarray(wg, dtype=np.float32)
    wu = np.asarray(wu, dtype=np.float32)
    wd = np.asarray(wd, dtype=np.float32)

    in_maps, tok_lists, caps = _pack(xf, gate_w, wg, wu, wd)
    if caps not in _CACHE:
        _CACHE[caps] = _build_nc(caps)
    nc = _CACHE[caps]
    out1 = np.zeros((T, H), np.float32)
    out2 = np.zeros((T, H), np.float32)

    res = run_bass_kernel_spmd(nc, in_maps, core_ids=list(range(N_CORES)))
    _CACHE["last_in_maps"] = in_maps
    _CACHE["last_caps"] = caps
    _CACHE["nc"] = nc

    for c in range(N_CORES):
        yc = res.results[c]["y"].astype(np.float32)        # [EPC, H, C] (yT)
        for j in range(EPC):
            toks, ranks, wts, n_dev, e = tok_lists[c][j]
            yr = np.ascontiguousarray(yc[j, :, :n_dev].T)
            sel1 = ranks[:n_dev] == 0
            sel2 = ~sel1
            out1[toks[:n_dev][sel1]] = yr[sel1]
            out2[toks[:n_dev][sel2]] = yr[sel2]
            if len(toks) > n_dev:                          # capacity overflow
                extra = toks[n_dev:]
                yextra = _expert_np(xf[extra], wts[n_dev:], wg[e], wu[e], wd[e])
                r = ranks[n_dev:]
                out1[extra[r == 0]] = yextra[r == 0]
                out2[extra[r == 1]] = yextra[r == 1]

    out = (out1 + out2).reshape(B, S, H)
    return out.astype(in_dtype, copy=False)



# revision 12
# speedup vs baseline: 1.3942x; 1.3942x over previous
"""Expert-parallel MoE FFN (ChronosMOEFeedForward) for 8 Trainium2 cores.

Strategy (sharding_hint: expert-parallel):
  - Router (softmax + top-2 over E=16 experts) computed on host in fp32 —
    top-k decisions must match the fp32 reference's ordering, and the router
    GEMM is ~0.1% of total FLOPs.
  - The 16 experts are sharded 2-per-core across 8 cores. Tokens routed to
    each expert are gathered on host (the "all-to-all dispatch"), padded to a
    fixed capacity C, and shipped transposed as [H, C] so the device GEMM
    chain needs no on-device transposes.
  - Per core the device computes, per expert e:
        gT = Wg[e].T @ XeT           [I, C]   (bf16 inputs, fp32 PSUM accum)
        uT = Wu[e].T @ XeT           [I, C]
        aT = silu(gT) * uT * w_bcast [I, C]   (combine weight broadcast on-chip)
        yT = Wd[e].T @ aT            [H, C]   (tokens stay on the free dim)
  - Host scatters each expert's y rows back to the owning tokens ("combine").
    A token's two expert contributions land in two disjoint slot arrays
    (top-1 slot, top-2 slot), so the combine is collision-free fancy
    indexing plus one add — no np.add.at.

Each PSUM group is computed with its k-tiles contiguous (all 16/8
accumulating matmuls back-to-back into one bank) — consecutive matmuls that
alternate PSUM banks measure ~40ns/MM slower on HW than k-contiguous
streams. The 8-deep PSUM pool overlaps each group's eviction with the next
group's matmuls, and the wg/wu halves share a 2-slot pool so the next
expert's weights stream in while the current expert computes. y returns as
bf16 to halve the output DMA.

The dense reference formulation computes all 16 experts for every token;
routed top-2 computes only 2 — an 8x FLOP reduction, plus bf16 matmuls with
fp32 PSUM accumulation.
"""

import numpy as np
import ml_dtypes

import concourse.mybir as mybir
import concourse.tile as tile
from concourse import bacc
from concourse.bass_utils import run_bass_kernel_spmd

# Problem shapes (hardcoded per contract).
H = 2048        # hidden size
I = 1024        # moe intermediate size
E = 16          # num experts
TOPK = 2
B, S = 4, 1024
T = B * S       # 4096 tokens
N_CORES = 8
EPC = E // N_CORES  # experts per core = 2
# Per-slot token capacities: each core gets one "heavy" expert (slot 0) and
# one "light" expert (slot 1). The host assigns the 8 heaviest-loaded experts
# to slot 0. Mean load is 512 +- ~22, so the light half of the experts almost
# always fits in 512; the few tokens that overflow fall back to exact numpy.
CAPS = (640, 512)
C = CAPS[0]      # max capacity (DRAM params are padded to this)
NCHS = (320, 256)  # free-dim chunk per slot for the g/u matmuls (2 chunks)

BF16 = ml_dtypes.bfloat16

KT_H = H // 128  # 16 k-tiles over H
MT_I = I // 128  # 8 m-tiles over I
KT_I = I // 128  # 8 k-tiles over I

# Split-precision fp8 phase A: host ships wg/wu and x as fp8e4 high+low
# pairs; the device computes the three significant half-products
# (wh*xh + wl*xh + wh*xl) with DoubleRow matmuls (2 fp8 MACs/cell/cycle),
# cutting phase-A PE cycles ~15% vs bf16 while keeping ~bf16 accuracy
# (the dropped wl*xl term is ~0.4% of each product). Phase B stays bf16.
# Measured on HW: numerically correct (rel err 6.0e-3) but NOT faster —
# DoubleRow ran at 1.0 cycles/row here (no 2x packing win), so the extra
# half-product stream made phase A ~50% slower. Kept for reference; off.
FP8A = False
F8 = ml_dtypes.float8_e4m3  # IEEE e4m3 (bias 7, +-240) = TRN FP8_EXP4
WSCALE = 64.0  # weights ~0.02*N(0,1) sit in e4m3 subnormals unless scaled

_CACHE = {}


def _build_nc(caps=CAPS, loop_r=None, internal=False):
    """Build the per-core Bass module (SPMD: all cores run this program).

    caps: per-slot token capacities. kernel() derives them from the actual
    routing (phase A streams tokens on the matmul free dim, so capacity is
    not 128-quantized there) and caches one compiled module per caps value.
    loop_r/internal are for the timing harness only: Internal DRAM I/O (no
    host transfers) with the body repeated loop_r times on-device.
    """
    import contextlib

    nc = bacc.Bacc(None, target_bir_lowering=False)
    f32 = mybir.dt.float32
    bf16 = mybir.dt.bfloat16
    f8 = mybir.dt.float8e4

    def par(name, shape, dt):
        if internal:
            return nc.dram_tensor(name, shape, dt)
        return nc.declare_dram_parameter(name, shape, dt, isOutput=False)

    if FP8A:
        ghl = par("ghl", [EPC, 2 * H, I], f8)   # (wh_t, wl_t) k-tile pairs
        gh = par("gh", [EPC, H, I], f8)
        uhl = par("uhl", [EPC, 2 * H, I], f8)
        uh = par("uh", [EPC, H, I], f8)
        xhh = par("xhh", [EPC, 2 * H, C], f8)   # xh k-tiles duplicated
        xl = par("xl", [EPC, H, C], f8)
        ins = (ghl, gh, uhl, uh, xhh, xl)
    else:
        xg = par("xg", [EPC, H, C], bf16)
        gww = par("gww", [EPC, H, I], bf16)
        uww = par("uww", [EPC, H, I], bf16)
        ins = (xg, gww, uww)
    wdp = par("wdp", [EPC, I, H], bf16)
    wtv = par("wtv", [EPC, C], f32)
    if internal:
        y = nc.dram_tensor("y", [EPC, H, C], bf16)
        done = nc.declare_dram_parameter("done", [1, 1], f32, isOutput=True)
    else:
        y = nc.declare_dram_parameter("y", [EPC, H, C], bf16, isOutput=True)

    with tile.TileContext(nc) as tc:
        with (
            tc.tile_pool(name="wpool", bufs=2) as wpool,   # wg/wu halves share slots
            tc.tile_pool(name="xpool", bufs=1) as xpool,
            tc.tile_pool(name="wdpool", bufs=1) as wdpool,
            tc.tile_pool(name="apool", bufs=1) as apool,
            tc.tile_pool(name="small", bufs=2) as small,
            tc.tile_pool(name="yp", bufs=4) as yp,
            tc.tile_pool(name="ps", bufs=8, space="PSUM") as ps,
        ):
            const = small.tile([1, 128], f32, tag="ones")
            nc.any.memset(const[:], 1.0)

            loop_cm = (
                tc.For_i(0, loop_r, 1) if loop_r else contextlib.nullcontext()
            )
            with loop_cm:
                if FP8A:
                    _emit_body_fp8(nc, tc, caps, ins, wdp, wtv, y, const,
                                   wpool, xpool, wdpool, apool, small, yp, ps)
                else:
                    _emit_body(nc, tc, caps, ins[0], ins[1], ins[2], wdp, wtv,
                               y, const, wpool, xpool, wdpool, apool, small,
                               yp, ps)

            if internal:
                dn = small.tile([1, 1], f32, tag="done")
                nc.any.memset(dn[:], 1.0)
                nc.sync.dma_start(out=done[:], in_=dn[:])

    nc.compile()
    return nc


def _emit_body_fp8(nc, tc, caps, ins, wdp, wtv, y, const,
                   wpool, xpool, wdpool, apool, small, yp, ps):
    """Phase A with split-precision fp8 DoubleRow; phase B bf16 (as before).

    Per (matrix, m-tile, chunk) the PSUM group accumulates 24 DoubleRow MMs:
    16 over (wh,wl)-pair k-tiles vs duplicated xh, then 8 over wh-pair
    k-tiles vs xl pairs — computing (wh+wl)*xh + wh*xl ~ w*x. Weights are
    pre-scaled by WSCALE on host (e4m3 range); sigmoid descales via the
    activation's scale input and the combine weights absorb 1/WSCALE^2.
    """
    f32 = mybir.dt.float32
    bf16 = mybir.dt.bfloat16
    f8 = mybir.dt.float8e4
    DR = mybir.MatmulPerfMode.DoubleRow
    ghl, gh, uhl, uh, xhh, xl = ins
    for e in range(EPC):
        Ce = caps[e]
        NCH = Ce if Ce <= 512 else (Ce + 1) // 2
        wt_sb = small.tile([1, C], f32, tag="wt")
        nc.sync.dma_start(out=wt_sb[:, :Ce], in_=wtv[e][None, :Ce])
        # g-side streams + x streams, in first-use order
        ghl_sb = wpool.tile([128, 2 * KT_H, I], f8, tag="whl")
        xhh_sb = xpool.tile([128, 2 * KT_H, C], f8, tag="xhh")
        for ko in range(2 * KT_H):
            nc.sync.dma_start(
                out=ghl_sb[:, ko, :], in_=ghl[e, ko * 128 : (ko + 1) * 128, :]
            )
            nc.sync.dma_start(
                out=xhh_sb[:, ko, :Ce],
                in_=xhh[e, ko * 128 : (ko + 1) * 128, :Ce],
            )
        gh_sb = wpool.tile([128, KT_H, I], f8, tag="wh8")
        xl_sb = xpool.tile([128, KT_H, C], f8, tag="xl8")
        for ko in range(KT_H):
            nc.sync.dma_start(
                out=gh_sb[:, ko, :], in_=gh[e, ko * 128 : (ko + 1) * 128, :]
            )
            nc.sync.dma_start(
                out=xl_sb[:, ko, :Ce], in_=xl[e, ko * 128 : (ko + 1) * 128, :Ce]
            )
        uhl_sb = wpool.tile([128, 2 * KT_H, I], f8, tag="whl")
        for ko in range(2 * KT_H):
            nc.sync.dma_start(
                out=uhl_sb[:, ko, :], in_=uhl[e, ko * 128 : (ko + 1) * 128, :]
            )
        uh_sb = wpool.tile([128, KT_H, I], f8, tag="wh8")
        for ko in range(KT_H):
            nc.sync.dma_start(
                out=uh_sb[:, ko, :], in_=uh[e, ko * 128 : (ko + 1) * 128, :]
            )
        wd_sb = wdpool.tile([128, KT_I, H], bf16, tag="wd")
        for ko in range(KT_I):
            nc.sync.dma_start(
                out=wd_sb[:, ko, :], in_=wdp[e, ko * 128 : (ko + 1) * 128, :]
            )

        sg_sb = apool.tile([128, MT_I, C], bf16, tag="sg")
        a_sb = apool.tile([128, MT_I, C], bf16, tag="a")
        wbc_sb = small.tile([128, C], bf16, tag="wbc")

        for c0 in range(0, Ce, NCH):
            w = min(NCH, Ce - c0)
            pw = ps.tile([128, 512], f32, tag="ps")
            nc.tensor.matmul(
                pw[:, :w], lhsT=const[:], rhs=wt_sb[:, c0 : c0 + w],
                start=True, stop=True,
            )
            nc.vector.tensor_copy(wbc_sb[:, c0 : c0 + w], pw[:, :w])

        # ---- phase A (fp8 DoubleRow, k-contiguous per psum group)
        for mat in range(2):
            whl_sb = ghl_sb if mat == 0 else uhl_sb
            wh8_sb = gh_sb if mat == 0 else uh_sb
            for m in range(MT_I):
                for c0 in range(0, Ce, NCH):
                    w = min(NCH, Ce - c0)
                    pt_t = ps.tile([128, 512], f32, tag="ps", name=f"ps_{m}_{c0}")
                    for t in range(KT_H):
                        nc.tensor.matmul(
                            pt_t[:, :w],
                            lhsT=whl_sb[:, 2 * t : 2 * t + 2, m * 128 : (m + 1) * 128],
                            rhs=xhh_sb[:, 2 * t : 2 * t + 2, c0 : c0 + w],
                            start=(t == 0),
                            stop=False,
                            perf_mode=DR,
                        )
                    for t in range(KT_H // 2):
                        nc.tensor.matmul(
                            pt_t[:, :w],
                            lhsT=wh8_sb[:, 2 * t : 2 * t + 2, m * 128 : (m + 1) * 128],
                            rhs=xl_sb[:, 2 * t : 2 * t + 2, c0 : c0 + w],
                            start=False,
                            stop=(t == KT_H // 2 - 1),
                            perf_mode=DR,
                        )
                    pt = pt_t[:, :w]
                    if mat == 0:
                        sig = small.tile([128, NCH], bf16, tag="sig")
                        nc.scalar.activation(
                            sig[:, :w], pt, mybir.ActivationFunctionType.Sigmoid,
                            scale=1.0 / WSCALE,
                        )
                        nc.vector.tensor_mul(sg_sb[:, m, c0 : c0 + w], sig[:, :w], pt)
                    else:
                        tmp = small.tile([128, NCH], bf16, tag="tmp")
                        nc.vector.tensor_mul(tmp[:, :w], sg_sb[:, m, c0 : c0 + w], pt)
                        nc.vector.tensor_mul(
                            a_sb[:, m, c0 : c0 + w], tmp[:, :w], wbc_sb[:, c0 : c0 + w]
                        )

        # ---- phase B (bf16, unchanged)
        bchunks = [(0, Ce)] if Ce <= 512 else [(0, NCH), (NCH, Ce - NCH)]
        for m in range(H // 128):
            yt = yp.tile([128, C], bf16, tag="ysb", name=f"yt_{m % 4}")
            for c0, w in bchunks:
                bpt = ps.tile([128, 512], f32, tag="ps", name=f"bps_{m % 4}_{c0}")
                for k in range(KT_I):
                    nc.tensor.matmul(
                        bpt[:, :w],
                        lhsT=wd_sb[:, k, m * 128 : (m + 1) * 128],
                        rhs=a_sb[:, k, c0 : c0 + w],
                        start=(k == 0),
                        stop=(k == KT_I - 1),
                    )
                nc.vector.tensor_copy(yt[:, c0 : c0 + w], bpt[:, :w])
            nc.sync.dma_start(
                out=y[e, m * 128 : (m + 1) * 128, :Ce], in_=yt[:, :Ce]
            )


def _emit_body(nc, tc, caps, xg, gww, uww, wdp, wtv, y, const,
               wpool, xpool, wdpool, apool, small, yp, ps):
    f32 = mybir.dt.float32
    bf16 = mybir.dt.bfloat16
    if True:
        if True:
            for e in range(EPC):
                Ce = caps[e]
                # one free-dim chunk if it fits a PSUM bank, else an even split
                NCH = Ce if Ce <= 512 else (Ce + 1) // 2
                # DMA issue order = need order: combine weights (tiny, feeds
                # the first PE instr group placed later), wg+xg k-tiles
                # interleaved (phase A ramp), then wu, then wd (phase B).
                wt_sb = small.tile([1, C], f32, tag="wt")
                nc.sync.dma_start(out=wt_sb[:, :Ce], in_=wtv[e][None, :Ce])
                gw_sb = wpool.tile([128, KT_H, I], bf16, tag="guw")
                xg_sb = xpool.tile([128, KT_H, C], bf16, tag="xg")
                for ko in range(KT_H):
                    nc.sync.dma_start(
                        out=gw_sb[:, ko, :], in_=gww[e, ko * 128 : (ko + 1) * 128, :]
                    )
                    nc.sync.dma_start(
                        out=xg_sb[:, ko, :Ce],
                        in_=xg[e, ko * 128 : (ko + 1) * 128, :Ce],
                    )
                uw_sb = wpool.tile([128, KT_H, I], bf16, tag="guw")
                for ko in range(KT_H):
                    nc.sync.dma_start(
                        out=uw_sb[:, ko, :], in_=uww[e, ko * 128 : (ko + 1) * 128, :]
                    )
                wd_sb = wdpool.tile([128, KT_I, H], bf16, tag="wd")
                for ko in range(KT_I):
                    nc.sync.dma_start(
                        out=wd_sb[:, ko, :], in_=wdp[e, ko * 128 : (ko + 1) * 128, :]
                    )

                sg_sb = apool.tile([128, MT_I, C], bf16, tag="sg")
                a_sb = apool.tile([128, MT_I, C], bf16, tag="a")
                wbc_sb = small.tile([128, C], bf16, tag="wbc")

                # broadcast combine weights across partitions via outer
                # product ones[128] x wt[C] -> wbc[128, C]; runs inside the
                # initial DMA ramp (wt is the first DMA issued) before any
                # g-group claims a psum bank
                for c0 in range(0, Ce, NCH):
                    w = min(NCH, Ce - c0)
                    pw = ps.tile([128, 512], f32, tag="ps")
                    nc.tensor.matmul(
                        pw[:, :w],
                        lhsT=const[:],
                        rhs=wt_sb[:, c0 : c0 + w],
                        start=True,
                        stop=True,
                    )
                    nc.vector.tensor_copy(wbc_sb[:, c0 : c0 + w], pw[:, :w])

                # ---- phase A: gT/uT, one k-contiguous psum group at a time.
                # Consecutive MMs that alternate psum banks pay ~40ns each on
                # HW (measured); k-contiguous streams run at ~ideal, and the
                # 8-deep psum pool still overlaps each group's eviction with
                # the next group's MMs.
                for mat in range(2):  # 0: g (silu), 1: u (mul + weight)
                    w_sb = gw_sb if mat == 0 else uw_sb
                    for m in range(MT_I):
                        for c0 in range(0, Ce, NCH):
                            w = min(NCH, Ce - c0)
                            pt_t = ps.tile([128, 512], f32, tag="ps", name=f"ps_{m}_{c0}")
                            for k in range(KT_H):
                                nc.tensor.matmul(
                                    pt_t[:, :w],
                                    lhsT=w_sb[:, k, m * 128 : (m + 1) * 128],
                                    rhs=xg_sb[:, k, c0 : c0 + w],
                                    start=(k == 0),
                                    stop=(k == KT_H - 1),
                                )
                            pt = pt_t[:, :w]
                            if mat == 0:
                                # silu(g) = g * sigmoid(g)
                                sig = small.tile([128, NCH], bf16, tag="sig")
                                nc.scalar.activation(
                                    sig[:, :w], pt, mybir.ActivationFunctionType.Sigmoid
                                )
                                nc.vector.tensor_mul(
                                    sg_sb[:, m, c0 : c0 + w], sig[:, :w], pt
                                )
                            else:
                                tmp = small.tile([128, NCH], bf16, tag="tmp")
                                nc.vector.tensor_mul(
                                    tmp[:, :w], sg_sb[:, m, c0 : c0 + w], pt
                                )
                                nc.vector.tensor_mul(
                                    a_sb[:, m, c0 : c0 + w],
                                    tmp[:, :w],
                                    wbc_sb[:, c0 : c0 + w],
                                )

                # ---- phase B: yT = Wd.T @ a   [H, Ce] — tokens stay on the
                # free dim, so only the exact Ce columns are streamed (no
                # 128-row quantization like the y = a.T @ Wd layout)
                bchunks = (
                    [(0, Ce)] if Ce <= 512 else [(0, NCH), (NCH, Ce - NCH)]
                )
                for m in range(H // 128):  # m-tiles over H, k-contiguous
                    yt = yp.tile([128, C], bf16, tag="ysb", name=f"yt_{m % 4}")
                    for c0, w in bchunks:
                        bpt = ps.tile(
                            [128, 512], f32, tag="ps", name=f"bps_{m % 4}_{c0}"
                        )
                        for k in range(KT_I):
                            nc.tensor.matmul(
                                bpt[:, :w],
                                lhsT=wd_sb[:, k, m * 128 : (m + 1) * 128],
                                rhs=a_sb[:, k, c0 : c0 + w],
                                start=(k == 0),
                                stop=(k == KT_I - 1),
                            )
                        nc.vector.tensor_copy(yt[:, c0 : c0 + w], bpt[:, :w])
                    nc.sync.dma_start(
                        out=y[e, m * 128 : (m + 1) * 128, :Ce],
                        in_=yt[:, :Ce],
                    )


def _route(xf, gate_w):
    """Top-2 routing, mirroring the fp32 reference semantics exactly."""
    logits = xf @ gate_w.T.astype(np.float32)          # [T, E]
    logits -= logits.max(axis=-1, keepdims=True)
    scores = np.exp(logits)
    scores /= scores.sum(axis=-1, keepdims=True)
    i1 = scores.argmax(axis=-1)
    s1 = scores[np.arange(T), i1]
    masked = scores.copy()
    masked[np.arange(T), i1] = -np.inf
    i2 = masked.argmax(axis=-1)
    s2 = scores[np.arange(T), i2]
    denom = s1 + s2 + 1e-20
    return i1, s1 / denom, i2, s2 / denom


def _expert_np(xrows, wts, wg_e, wu_e, wd_e):
    """Exact fp32 fallback for capacity-overflow tokens (rare)."""
    g = xrows @ wg_e
    u = xrows @ wu_e
    a = (g / (1.0 + np.exp(-g))) * u * wts[:, None]
    return a @ wd_e


def _pack(xf, gate_w, wg, wu, wd):
    """Route + gather + pack per-core device inputs.

    Experts are assigned to (core, slot) by load: the 8 heaviest go to the
    C=640 slot 0, the 8 lightest to the C=512 slot 1. The assignment is pure
    host-side data placement — the SPMD program is identical on every core.
    """
    i1, w1, i2, w2 = _route(xf, gate_w)
    per_e = []
    for e in range(E):
        l1 = np.nonzero(i1 == e)[0]
        l2 = np.nonzero(i2 == e)[0]
        toks = np.concatenate([l1, l2])
        wts = np.concatenate([w1[l1], w2[l2]])
        ranks = np.concatenate(
            [np.zeros(len(l1), np.int8), np.ones(len(l2), np.int8)]
        )
        per_e.append((toks, ranks, wts))
    loads = [len(pe[0]) for pe in per_e]
    order = np.argsort([-n for n in loads], kind="stable")
    # exact capacities from this routing: slot 0 covers the heaviest expert
    # (up to the 640 DRAM padding), slot 1 stays at <=512 so its phase-B tile
    # count stays at 4; the rare overflow tokens go to the exact numpy path
    caps = (
        min(loads[order[0]], CAPS[0]),
        min(max(loads[order[N_CORES]], 128), CAPS[1]),
    )

    in_maps = []
    tok_lists = []
    for c in range(N_CORES):
        wtc = np.zeros((EPC, C), np.float32)
        core_toks = []
        experts = [int(order[c]), int(order[2 * N_CORES - 1 - c])]
        if FP8A:
            ghl = np.zeros((EPC, 2 * H, I), F8)
            ghp = np.zeros((EPC, H, I), F8)
            uhl = np.zeros((EPC, 2 * H, I), F8)
            uhp = np.zeros((EPC, H, I), F8)
            xhh = np.zeros((EPC, 2 * H, C), F8)
            xlp = np.zeros((EPC, H, C), F8)
        else:
            xgc = np.zeros((EPC, H, C), BF16)
        for j in range(EPC):
            e = experts[j]
            toks, ranks, wts = per_e[e]
            n_dev = min(len(toks), caps[j])
            if FP8A:
                xe = np.ascontiguousarray(xf[toks[:n_dev]].T)      # [H, n]
                xh = xe.astype(F8)
                xr = (xe - xh.astype(np.float32)).astype(F8)
                xhh[j, :, :n_dev] = _ilv(xh, xh)
                xlp[j, :, :n_dev] = xr
                for dsthl, dsth, wmat in ((ghl, ghp, wg), (uhl, uhp, wu)):
                    ws = wmat[e] * WSCALE                          # [H, I]
                    wh = ws.astype(F8)
                    wl = (ws - wh.astype(np.float32)).astype(F8)
                    dsthl[j] = _ilv(wh, wl)
                    dsth[j] = wh
                wtc[j, :n_dev] = wts[:n_dev] / (WSCALE * WSCALE)
            else:
                xgc[j, :, :n_dev] = xf[toks[:n_dev]].T.astype(BF16)
                wtc[j, :n_dev] = wts[:n_dev]
            core_toks.append((toks, ranks, wts, n_dev, e))
        tok_lists.append(core_toks)
        im = {"wdp": wd[experts].astype(BF16), "wtv": wtc}
        if FP8A:
            im.update(ghl=ghl, gh=ghp, uhl=uhl, uh=uhp, xhh=xhh, xl=xlp)
        else:
            im.update(
                xg=xgc,
                gww=wg[experts].astype(BF16),
                uww=wu[experts].astype(BF16),
            )
        in_maps.append(im)
    return in_maps, tok_lists, caps


def _ilv(a, b):
    """Interleave the 128-row k-tiles of a and b: [H', X] x2 -> [2H', X]."""
    kt = a.shape[0] // 128
    st = np.stack([a.reshape(kt, 128, -1), b.reshape(kt, 128, -1)], axis=1)
    return st.reshape(2 * a.shape[0], a.shape[1])


def kernel(x, gate_w, wg, wu, wd):
    # inputs may arrive as jax arrays (immutable) — normalize to numpy first
    x = np.asarray(x)
    gate_w = np.asarray(gate_w, dtype=np.float32)
    in_dtype = x.dtype
    xf = np.ascontiguousarray(x.reshape(T, H), dtype=np.float32)
    wg = np.asarray(wg, dtype=np.float32)
    wu = np.asarray(wu, dtype=np.float32)
    wd = np.asarray(wd, dtype=np.float32)

    in_maps, tok_lists, caps = _pack(xf, gate_w, wg, wu, wd)
    if caps not in _CACHE:
        _CACHE[caps] = _build_nc(caps)
    nc = _CACHE[caps]
    out1 = np.zeros((T, H), np.float32)
    out2 = np.zeros((T, H), np.float32)

    res = run_bass_kernel_spmd(nc, in_maps, core_ids=list(range(N_CORES)))
    _CACHE["last_in_maps"] = in_maps
    _CACHE["last_caps"] = caps
    _CACHE["nc"] = nc

    for c in range(N_CORES):
        yc = res.results[c]["y"].astype(np.float32)        # [EPC, H, C] (yT)
        for j in range(EPC):
            toks, ranks, wts, n_dev, e = tok_lists[c][j]
            yr = np.ascontiguousarray(yc[j, :, :n_dev].T)
            sel1 = ranks[:n_dev] == 0
            sel2 = ~sel1
            out1[toks[:n_dev][sel1]] = yr[sel1]
            out2[toks[:n_dev][sel2]] = yr[sel2]
            if len(toks) > n_dev:                          # capacity overflow
                extra = toks[n_dev:]
                yextra = _expert_np(xf[extra], wts[n_dev:], wg[e], wu[e], wd[e])
                r = ranks[n_dev:]
                out1[extra[r == 0]] = yextra[r == 0]
                out2[extra[r == 1]] = yextra[r == 1]

    out = (out1 + out2).reshape(B, S, H)
    return out.astype(in_dtype, copy=False)

